# revision 41
# baseline (speedup 1.0000x reference)
"""Trainium2 Bass kernel for nn_DreamGraphReasoner (8 NeuronCores).

Model (per batch element):
  x = mean(what, action, result)                  (N=1024 nodes, D=512)
  3 hops of sparse graph attention; per hop:
      Q=xWq+bq, K=xWk+bk, V=xWv+bv
      attn = softmax(mask(QK^T/sqrt(D)))          mask: same-step cross-dream
      x += relu((attn V) W_hop[h] + b_hop[h])           + next-step same-dream
  out = relu(mean_nodes(x) @ W_agg1 + b_agg1) @ W_agg2 + b_agg2

Distribution: data-parallel over batch B=16 -> 2 batch elements per core,
concatenated into one 2048-node axis on each core; weights replicated.

Kernel design (fp8 DoubleRow rewrite of the f32r windowed-attention kernel):
  * Step-major node permutation (node = step*G + dream): the edge mask is
    block-diagonal (16x16 per step, minus identity) plus a +16
    super-diagonal, so attention runs on 8 windows of 256 queries x 272
    keys instead of dense 2048^2.
  * Fused QK projection: scores(q,k) = (x@M + w0).x_k with M = Wq Wk^T
    (bk cancels in softmax; w0 = Wk bq is folded in as a bias).
  * All hop GEMMs run as fp8e4m3 DoubleRow matmuls (two 128-deep k-tiles
    per instruction at ~0.5 cyc/row): measured ~4.6x f32r throughput on
    hardware. Scale management: weights x32, M x64, G' x4, attended x8;
    the inverse scales fold into PSUM-evacuation activation scales for
    free. The residual stream master is bf16 (in-place fp8 accumulation
    measured over the 2e-2 error budget); a packed fp8 working copy
    (d-chunk pairs interleaved) feeds the PE.
  * The mask add is a diag(4)xfp8 matmul into the scores PSUM (mask value
    -240 = most-negative finite e4m3; after the exp its weight underflows
    fp8 to 0). exp runs on ACT with fused row-sum accumulation; softmax
    normalization multiplies in the f32 reciprocal (reciprocal_approx_fast
    on DVE) BEFORE the single fp8 rounding of attn (best accuracy order).
  * attn transposes go through the PE in fp8 (4 per window batched into
    one PSUM tile, one evacuation copy); attended = attn@V contracts both
    key blocks in one DoubleRow matmul; the +16 temporal key block only
    feeds the last 16 queries so its transpose/matmul are 16-wide.
  * Inputs stream in as bf16 (half the DMA of f32; x0 error ~0.1%), the
    final-MLP weights as bf16, Wq/Wk as bf16 (M is rounded to fp8 anyway),
    Wv/W_hop as pre-scaled fp8 — ~10.4 MB HBM traffic per core total.
  * 4-deep software pipeline over windows: front(w) = V/G'/scores on PE,
    transposes(w-2), attend+residual(w-3) - softmax latency (ACT/DVE)
    hides under the next windows' PE work. Hardware wants far more slack
    than the cost-model sim predicts (depth 3->4 sim'd -9% but measured
    -25%); depth 5 regresses (PSUM bank rotation limit). The scores PSUM
    shares the single 8-buf PSUM rotation to maximize bank slots.
  * Engine assignment measured on hardware (micro-slopes): ACT PSUM-
    evacuations are ~3x cheaper than DVE and GPSIMD cannot touch PSUM
    and is 2-8x slower than DVE on wide SBUF ops. So: ACT takes G'/half-
    V/exp/relu evacuations, DVE takes attended/half-V evacuations plus
    the SBUF-side normalize/transpose-copy/fp8-cast ops, GPSIMD only the
    bf16 master accumulate. Node-sums for the final mean ride accum_out
    of the load/relu evacuations (no separate reduction pass).
  * End-to-end scale-relative error vs the fp32 jax reference ~1.1e-2
    (threshold 2e-2), dominated by the fp8 quantization of attn/V/W_hop.
"""

import os
import sys
from contextlib import ExitStack

for _p in ("/opt/trn_rl_repo", "/root/.axon_site/_ro/trn_rl_repo"):
    if os.path.isdir(_p) and _p not in sys.path:
        sys.path.insert(0, _p)

import numpy as np
import ml_dtypes

import concourse.bass as bass
import concourse.mybir as mybir
import concourse.tile as tile
from concourse import bacc
from concourse.bass_utils import run_bass_kernel_spmd

G, L, B, D, H = 16, 64, 16, 512, 3
N_CORES = 8
BPC = B // N_CORES          # batch elems per core = 2
N = G * L                   # nodes per batch elem = 1024
NT = BPC * N                # nodes per core = 2048
PAD = 16                    # padding keys for the last temporal window
NTP = NT + PAD
W = 256                     # queries per attention window (16 steps)
KW = W + 16                 # keys per window (incl. next-step diagonal)
NWIN = NT // W              # 8 windows
KT = D // 128               # 4 k-tiles over D
DT = mybir.dt.float32
BF = mybir.dt.bfloat16
F8 = mybir.dt.float8e4
DR = mybir.MatmulPerfMode.DoubleRow
SCALE = 1.0 / float(np.sqrt(D))

WS = 32.0      # Wv / W_hop fp8 pre-scale
MS = 64.0      # M fp8 scale
GS = 4.0       # G' fp8 scale
AS = 8.0       # attended fp8 scale
MASKVAL = -240.0  # most-negative finite e4m3


def _r2(ap, inner):
    """[128, 2*inner] tile AP -> [128, 2, inner] DoubleRow operand AP."""
    return ap.rearrange("p (i n) -> p i n", i=2)


def build_masks() -> np.ndarray:
    """Additive masks for one 256-query window, per 128-query subtile.

    Returns (3, 128, KW): [sub0, sub1, sub1_last_window]. Rows are
    window-local queries; columns are window-local keys [0, 272).
    """
    m = np.full((2, 128, KW), MASKVAL, np.float32)
    for sub in range(2):
        for ql in range(128):
            q = sub * 128 + ql
            t, g = divmod(q, G)
            for h in range(G):
                if h != g:
                    m[sub, ql, t * G + h] = 0.0    # same step, other dream
            m[sub, ql, q + 16] = 0.0               # next step, same dream
    m_last = m[1].copy()
    m_last[:, W:] = MASKVAL   # final step of the batch has no next step
    return np.stack([m[0], m[1], m_last])


def build_module(rep: int = 1):
    nc = bacc.Bacc(None, target_bir_lowering=False)

    # inputs pre-permuted on host to step-major node order (contiguous DMA)
    what = nc.dram_tensor("what", [NT, D], BF, kind="ExternalInput")
    action = nc.dram_tensor("action", [NT, D], BF, kind="ExternalInput")
    result = nc.dram_tensor("result", [NT, D], BF, kind="ExternalInput")
    Wq = nc.dram_tensor("Wq", [D, D], BF, kind="ExternalInput")
    Wk = nc.dram_tensor("Wk", [D, D], BF, kind="ExternalInput")
    wv8d = nc.dram_tensor("wv8", [2, 128, 2, D], F8, kind="ExternalInput")
    wh8d = nc.dram_tensor("wh8", [H, 2, 128, 2, D], F8, kind="ExternalInput")
    w0sd = nc.dram_tensor("w0s", [D], DT, kind="ExternalInput")   # GS*(Wk@bq)
    bvd = nc.dram_tensor("bv", [D], DT, kind="ExternalInput")
    bhopd = nc.dram_tensor("b_hop", [H, D], DT, kind="ExternalInput")
    Wa1 = nc.dram_tensor("W_agg1", [D, 2 * D], BF, kind="ExternalInput")
    ba1 = nc.dram_tensor("b_agg1", [2 * D], DT, kind="ExternalInput")
    Wa2 = nc.dram_tensor("W_agg2", [2 * D, D], BF, kind="ExternalInput")
    ba2 = nc.dram_tensor("b_agg2", [D], DT, kind="ExternalInput")
    masksd = nc.dram_tensor("masks", [3, 128, KW], F8, kind="ExternalInput")
    identbd = nc.dram_tensor("identb", [128, 128], BF, kind="ExternalInput")
    ident8d = nc.dram_tensor("ident8", [128, 128], F8, kind="ExternalInput")
    idgs8d = nc.dram_tensor("idgs8", [128, 128], F8, kind="ExternalInput")
    out = nc.dram_tensor("out", [BPC, D], DT, kind="ExternalOutput")

    AF = mybir.ActivationFunctionType
    ALU = mybir.AluOpType

    with tile.TileContext(nc) as tc, ExitStack() as st:
        pp = st.enter_context(tc.tile_pool(name="persist", bufs=1))
        pld = st.enter_context(tc.tile_pool(name="ld", bufs=4))
        psm = st.enter_context(tc.tile_pool(name="sm", bufs=6))
        pat = st.enter_context(tc.tile_pool(name="attn", bufs=4))
        pgt = st.enter_context(tc.tile_pool(name="gt", bufs=2))
        pvb = st.enter_context(tc.tile_pool(name="vblk", bufs=7))
        pac = st.enter_context(tc.tile_pool(name="atc", bufs=3))
        pwh = st.enter_context(tc.tile_pool(name="whop", bufs=2))
        ppw = st.enter_context(tc.tile_pool(name="psw", bufs=8,
                                            space="PSUM"))

        # ---- identities + persistent activations first (loads gate PE) ----
        idtb = pp.tile([128, 128], BF, name="idtb", tag="idtb")
        nc.sync.dma_start(out=idtb, in_=identbd[:, :])
        idt8 = pp.tile([128, 128], F8, name="idt8", tag="idt8")
        nc.sync.dma_start(out=idt8, in_=ident8d[:, :])
        idgs8 = pp.tile([128, 128], F8, name="idgs8", tag="idgs8")
        nc.sync.dma_start(out=idgs8, in_=idgs8d[:, :])
        # bf16 residual master + packed fp8 working copy (padded for the
        # last window's temporal keys). Node-sums for the final mean ride
        # on accum_out of the load / relu evacuations into asb columns.
        xTb = [pp.tile([128, NT], BF, name=f"xTb{k}", tag=f"xTb{k}")
               for k in range(KT)]
        xT8 = [pp.tile([128, 2 * NTP], F8, name=f"xT8{p}", tag=f"xT8{p}")
               for p in range(2)]
        for p in range(2):
            for i in range(2):
                nc.vector.memset(
                    xT8[p][:, i * NTP + NT:(i + 1) * NTP], 0.0)
        ACC = 8 + rep * H * 2   # asb cols per batch elem: 8 load + 2/hop
        asb = [pp.tile([128, 2 * ACC], DT, name=f"asb{k}", tag=f"asb{k}")
               for k in range(KT)]

        def load_tile(i):
            """x = (what+action+result)/3 for node-tile i: 3 accumulated
            bf16 transpose-matmuls -> bf16 master (with node-sum accum)
            + fp8 working copy."""
            tw = pld.tile([128, D], BF, name="ldw", tag="ldw")
            ta = pld.tile([128, D], BF, name="lda", tag="lda")
            tr = pld.tile([128, D], BF, name="ldr", tag="ldr")
            nc.sync.dma_start(out=tw, in_=what[i * 128:(i + 1) * 128, :])
            nc.sync.dma_start(out=ta, in_=action[i * 128:(i + 1) * 128, :])
            nc.sync.dma_start(out=tr, in_=result[i * 128:(i + 1) * 128, :])
            for c in range(KT):
                pt = ppw.tile([128, 128], DT, name="psw", tag="psw")
                for j, t in enumerate((tw, ta, tr)):
                    nc.tensor.matmul(pt, t[:, c * 128:(c + 1) * 128], idtb,
                                     start=(j == 0), stop=(j == 2))
                nc.vector.tensor_scalar(
                    xTb[c][:, i * 128:(i + 1) * 128],
                    pt, 1.0 / 3.0, 0.0, ALU.mult, ALU.add,
                    accum_out=asb[c][:, (i // 8) * ACC + i % 8:
                                     (i // 8) * ACC + i % 8 + 1])
                nc.scalar.mul(
                    xT8[c // 2][:, (c % 2) * NTP + i * 128:
                                (c % 2) * NTP + (i + 1) * 128],
                    pt, 1.0 / 3.0)

        # ---- small per-partition constants ----
        w0s = pp.tile([128, KT], DT, name="w0s", tag="w0s")
        bvv = pp.tile([128, KT], DT, name="bvv", tag="bvv")
        bhv = pp.tile([128, H * KT], DT, name="bhv", tag="bhv")
        msk = [pp.tile([128, KW], F8, name=f"msk{j}", tag=f"msk{j}")
               for j in range(3)]
        wv8t = [pp.tile([128, 2 * D], F8, name=f"wv8{p}", tag=f"wv8{p}")
                for p in range(2)]
        m8 = [pp.tile([128, 2 * D], F8, name=f"m8{p}", tag=f"m8{p}")
              for p in range(2)]

        # ---- M = Wq @ Wk^T in bf16, rounded to fp8 (x64) ----
        with tc.tile_pool(name="wtrans", bufs=1) as pw:
            wq = [pw.tile([128, D], BF, name=f"wq{k}", tag=f"wq{k}")
                  for k in range(KT)]
            wk = [pw.tile([128, D], BF, name=f"wk{k}", tag=f"wk{k}")
                  for k in range(KT)]
            for k in range(KT):
                nc.sync.dma_start(out=wq[k], in_=Wq[k * 128:(k + 1) * 128, :])
                nc.sync.dma_start(out=wk[k], in_=Wk[k * 128:(k + 1) * 128, :])
            # input tiles for hop-0 windows 0-1 stream in behind the weights;
            # the M precompute keeps the PE busy while they load
            for i in range(5):
                load_tile(i)
            # remaining small constants queue behind the weights
            for p in range(2):
                nc.sync.dma_start(out=wv8t[p], in_=wv8d[p, :, :, :])
            for j in range(3):
                nc.sync.dma_start(out=msk[j], in_=masksd[j])
            for k in range(KT):
                nc.sync.dma_start(out=w0s[:, k:k + 1],
                                  in_=w0sd[k * 128:(k + 1) * 128])
                nc.sync.dma_start(out=bvv[:, k:k + 1],
                                  in_=bvd[k * 128:(k + 1) * 128])
            for h in range(H):
                for k in range(KT):
                    nc.sync.dma_start(out=bhv[:, h * KT + k:h * KT + k + 1],
                                      in_=bhopd[h, k * 128:(k + 1) * 128])
            wqT = [pw.tile([128, D], BF, name=f"wqT{k}", tag=f"wqT{k}")
                   for k in range(KT)]
            wkT = [pw.tile([128, D], BF, name=f"wkT{k}", tag=f"wkT{k}")
                   for k in range(KT)]
            for i in range(KT):
                for j in range(KT):
                    ptq = ppw.tile([128, 128], BF, name="psw", tag="psw")
                    nc.tensor.transpose(ptq, wq[i][:, j * 128:(j + 1) * 128],
                                        idtb)
                    nc.vector.tensor_copy(
                        out=wqT[j][:, i * 128:(i + 1) * 128], in_=ptq)
                    ptk = ppw.tile([128, 128], BF, name="psw", tag="psw")
                    nc.tensor.transpose(ptk, wk[i][:, j * 128:(j + 1) * 128],
                                        idtb)
                    nc.vector.tensor_copy(
                        out=wkT[j][:, i * 128:(i + 1) * 128], in_=ptk)
            # M[din, dout] = sum_c Wq[din, c] Wk[dout, c]; fp8 out x64
            for mt in range(KT):
                ps = ppw.tile([128, D], DT, name="psw", tag="psw")
                for k in range(KT):
                    nc.tensor.matmul(
                        ps, wqT[k][:, mt * 128:(mt + 1) * 128], wkT[k],
                        start=(k == 0), stop=(k == KT - 1))
                nc.scalar.mul(
                    m8[mt // 2][:, (mt % 2) * D:(mt % 2) * D + D], ps, MS)

        # final-MLP weights: pool opened after wtrans closes, reusing space
        pfin = st.enter_context(tc.tile_pool(name="fin", bufs=1))
        wa1 = [pfin.tile([128, 2 * D], BF, name=f"wa1{k}", tag=f"wa1{k}")
               for k in range(KT)]
        wa2 = [pfin.tile([128, D], BF, name=f"wa2{k}", tag=f"wa2{k}")
               for k in range(8)]
        b1b = pfin.tile([BPC, 2 * D], DT, name="b1b", tag="b1b")
        b2b = pfin.tile([BPC, D], DT, name="b2b", tag="b2b")

        def load_final_weights():
            for k in range(KT):
                nc.sync.dma_start(out=wa1[k],
                                  in_=Wa1[k * 128:(k + 1) * 128, :])
            for k in range(8):
                nc.sync.dma_start(out=wa2[k],
                                  in_=Wa2[k * 128:(k + 1) * 128, :])
            nc.sync.dma_start(out=b1b, in_=bass.AP(
                tensor=ba1, offset=0, ap=[[0, BPC], [1, 2 * D]]))
            nc.sync.dma_start(out=b2b, in_=bass.AP(
                tensor=ba2, offset=0, ap=[[0, BPC], [1, D]]))

        # ---- hops: software-pipelined window loop ----
        # Per pipeline step: emit the "front" of window (h, w) -- V blocks,
        # G' projection, scores+mask+exp -- then the transposes of window
        # (h, w-1) and the attend+residual of window (h, w-2). The softmax
        # of window w runs on ACT/DVE while the PE works on window w+1.
        hops = [hh % H for hh in range(rep * H)]
        gt8 = None
        vb8 = {}           # (hop-step, window) -> packed fp8 V tile
        aTc8 = [None, None]
        pa_pair = None     # window-pair attended PSUM tiles, one per dc
        wh_by_step = {}

        def v_block(hs, b):
            wv_, c = divmod(b, 2)
            if c == 0:
                vb8[(hs, wv_)] = pvb.tile([128, 2 * D], F8, name="vblk",
                                          tag="vblk")
            t = vb8[(hs, wv_)]
            ps = ppw.tile([128, D], DT, name="psw", tag="psw")
            for p in range(2):
                nc.tensor.matmul(
                    ps, _r2(xT8[p][:, :], NTP)[:, :, b * 128:(b + 1) * 128],
                    _r2(wv8t[p][:, :], D),
                    start=(p == 0), stop=(p == 1), perf_mode=DR)
            if b % 2 == 0:
                nc.scalar.mul(t[:, c * D:(c + 1) * D], ps, 1.0 / WS)
            else:
                nc.vector.tensor_scalar_mul(t[:, c * D:(c + 1) * D], ps,
                                            1.0 / WS)

        def emit_front(hs, h, w):
            q0 = w * W
            last = (w % (N // W) == N // W - 1)
            if hs == 0 and w >= 1:
                for i in (2 * w + 3, 2 * w + 4):
                    if i < NT // 128:
                        load_tile(i)
            if w == 0 and hs == min(1, rep * H - 1):
                load_final_weights()
            if w == 0:
                wh = [pwh.tile([128, 2 * D], F8, name=f"wh{p}", tag=f"wh{p}")
                      for p in range(2)]
                for p in range(2):
                    nc.sync.dma_start(out=wh[p], in_=wh8d[h, p, :, :, :])
                wh_by_step[hs] = wh
                for b in (0, 1, 2):
                    v_block(hs, b)
            else:
                v_block(hs, 2 * w + 1)
                if 2 * w + 2 < NT // 128:
                    v_block(hs, 2 * w + 2)
            # G'^T for a window PAIR (moving 512) computed at even windows
            nonlocal gt8
            if w % 2 == 0:
                gt8 = [pgt.tile([128, 2 * W * 2], F8, name=f"gt{p}",
                                tag=f"gt{p}") for p in range(2)]
                for mt in range(KT):
                    ps = ppw.tile([128, 512], DT, name="psw", tag="psw")
                    for p in range(2):
                        nc.tensor.matmul(
                            ps, _r2(m8[p][:, :], D)[:, :, mt * 128:(mt + 1) * 128],
                            _r2(xT8[p][:, :], NTP)[:, :, q0:q0 + 2 * W],
                            start=(p == 0), stop=(p == 1), perf_mode=DR)
                    nc.scalar.activation(
                        gt8[mt // 2][:, (mt % 2) * 2 * W:(mt % 2 + 1) * 2 * W],
                        ps, AF.Identity, bias=w0s[:, mt:mt + 1],
                        scale=GS / MS)
            # scores + mask (diag(GS) x fp8 mask matmul into the PSUM),
            # then exp on ACT (fused row-sum), reciprocal+normalize on DVE
            ex = [None, None]
            for sub in range(2):
                pss = ppw.tile([128, KW], DT, name="psw", tag="psw")
                qoff = (w % 2) * W + sub * 128
                for p in range(2):
                    nc.tensor.matmul(
                        pss, _r2(gt8[p][:, :], 2 * W)[:, :, qoff:qoff + 128],
                        _r2(xT8[p][:, :], NTP)[:, :, q0:q0 + KW],
                        start=(p == 0), stop=False, perf_mode=DR)
                mj = msk[2] if (sub == 1 and last) else msk[sub]
                nc.tensor.matmul(pss, idgs8, mj, start=False, stop=True)
                e_bf = psm.tile([128, KW], BF, name="ebf", tag="ebf")
                sm = psm.tile([128, 1], DT, name="sm", tag="sm")
                nc.scalar.activation(e_bf, pss, AF.Exp, bias=0.0,
                                     scale=SCALE / GS, accum_out=sm)
                rc = psm.tile([128, 1], DT, name="rc", tag="rc")
                nc.vector.reciprocal_approx_fast(rc, sm)
                e_n = psm.tile([128, KW], BF, name="en", tag="en")
                nc.vector.tensor_scalar_mul(e_n, e_bf, rc)
                ex[sub] = e_n
            return dict(hs=hs, h=h, w=w, q0=q0, last=last, ex=ex)

        def emit_transp(stt):
            hs, h, w, q0, last, ex = (stt[k] for k in
                                      ("hs", "h", "w", "q0", "last", "ex"))
            # transpose attn (bf16) -> fp8 aTk [keys, (kblock, 256 queries)];
            # 4 PE transposes batched into one PSUM tile, one converting copy
            pt8 = ppw.tile([128, 512], BF, name="psw", tag="psw")
            for c in range(2):
                for sub in range(2):
                    nc.tensor.transpose(
                        pt8[:, (c * 2 + sub) * 128:(c * 2 + sub + 1) * 128],
                        ex[sub][:, c * 128:(c + 1) * 128], idtb)
            aTk = pat.tile([128, 512], F8, name="aTk", tag="aTk")
            nc.vector.tensor_copy(out=aTk, in_=pt8)
            nch = 2 if last else 3
            if nch == 3:
                ptx = ppw.tile([128, 64], BF, name="psw", tag="psw")
                nc.tensor.transpose(
                    ptx[0:16, 0:64], ex[1][64:128, 256:272],
                    idtb[64:128, 64:128])
                aTkx = pat.tile([128, 16], F8, name="aTkx", tag="aTkx")
                nc.vector.tensor_copy(out=aTkx[0:16, 0:16],
                                      in_=ptx[0:16, 48:64])
                stt["aTkx"] = aTkx
            stt["aTk"] = aTk
            stt["nch"] = nch

        def emit_attend(stt):
            nonlocal aTc8, pa_pair
            hs, h, w, q0, last, aTk, nch = (stt[k] for k in
                                            ("hs", "h", "w", "q0", "last",
                                             "aTk", "nch"))
            if w % 2 == 0:
                aTc8 = [pac.tile([128, 2 * D], F8, name=f"aTc{p}",
                                 tag=f"aTc{p}") for p in range(2)]
                pa_pair = [ppw.tile([128, 2 * W], DT, name="psw", tag="psw")
                           for _ in range(KT)]
            # attended^T = V_window^T @ attn^T into the pair PSUM; one
            # (+bv, xAS) fp8 evacuation per dc per window PAIR
            for dc in range(KT):
                pa = pa_pair[dc][:, (w % 2) * W:(w % 2) * W + W]
                nc.tensor.matmul(
                    pa, _r2(vb8[(hs, w)][:, :], D)[:, :, dc * 128:(dc + 1) * 128],
                    _r2(aTk[:, :], W),
                    start=True, stop=(nch == 2), perf_mode=DR)
                if nch == 3:
                    vb_n = vb8[(hs, w + 1)]
                    nc.tensor.matmul(
                        pa[:, 240:256],
                        vb_n[0:16, dc * 128:(dc + 1) * 128],
                        stt["aTkx"][0:16, 0:16], start=False, stop=True)
                if w % 2 == 1:
                    nc.vector.tensor_scalar(
                        aTc8[dc // 2][:, (dc % 2) * D:(dc % 2 + 1) * D],
                        pa_pair[dc], AS, bvv[:, dc:dc + 1],
                        ALU.mult, ALU.add)
            # residual update of the finished 512-node chunk: relu evac with
            # free node-sum accumulation, then in-place fp8 accumulate
            if w % 2 == 1:
                ch = w // 2
                wh = wh_by_step[hs]
                for mt in range(KT):
                    ps = ppw.tile([128, 512], DT, name="psw", tag="psw")
                    for p in range(2):
                        nc.tensor.matmul(
                            ps, _r2(wh[p][:, :], D)[:, :, mt * 128:(mt + 1) * 128],
                            _r2(aTc8[p][:, :], D),
                            start=(p == 0), stop=(p == 1), perf_mode=DR)
                    rl = psm.tile([128, 512], DT, name="rl", tag="rl")
                    nc.scalar.activation(
                        rl, ps, AF.Relu,
                        bias=bhv[:, h * KT + mt:h * KT + mt + 1],
                        scale=1.0 / (WS * AS),
                        accum_out=asb[mt][:, (ch // 2) * ACC + 8 + hs * 2
                                          + ch % 2:
                                          (ch // 2) * ACC + 9 + hs * 2
                                          + ch % 2])
                    if hs != len(hops) - 1:
                        mb = xTb[mt][:, ch * 512:(ch + 1) * 512]
                        nc.gpsimd.tensor_add(mb, mb, rl)
                        nc.vector.tensor_copy(
                            out=xT8[mt // 2][:, (mt % 2) * NTP + ch * 512:
                                             (mt % 2) * NTP + (ch + 1) * 512],
                            in_=mb)

        states = []
        for hs, h in enumerate(hops):
            for w in range(NWIN):
                states.append(emit_front(hs, h, w))
                if len(states) >= 3:
                    emit_transp(states[-3])
                if len(states) >= 4:
                    emit_attend(states[-4])
        emit_transp(states[-2])
        emit_transp(states[-1])
        emit_attend(states[-3])
        emit_attend(states[-2])
        emit_attend(states[-1])

        # ---- final: agg = mean_nodes(x); 2-layer MLP in bf16 ----
        agg = [pfin.tile([128, BPC], BF, name=f"agg{k}", tag=f"agg{k}")
               for k in range(KT)]
        for k in range(KT):
            asum = psm.tile([128, BPC], DT, name="asum", tag="asum")
            for b_ in range(BPC):
                nc.vector.reduce_sum(asum[:, b_:b_ + 1],
                                     asb[k][:, b_ * ACC:(b_ + 1) * ACC],
                                     axis=mybir.AxisListType.X)
            nc.vector.tensor_scalar_mul(agg[k], asum, 1.0 / N)
        hdn = pfin.tile([BPC, 2 * D], BF, name="hdn", tag="hdn")
        for ch in range(2):
            ps = ppw.tile([128, 512], DT, name="psw", tag="psw")
            for k in range(KT):
                nc.tensor.matmul(ps[0:BPC, :], agg[k],
                                 wa1[k][:, ch * 512:(ch + 1) * 512],
                                 start=(k == 0), stop=(k == KT - 1))
            nc.vector.tensor_add(hdn[:, ch * 512:(ch + 1) * 512],
                                 ps[0:BPC, :], b1b[:, ch * 512:(ch + 1) * 512])
        nc.vector.tensor_scalar_max(hdn, hdn, 0.0)
        hT = pfin.tile([128, 2 * 8], BF, name="hT", tag="hT")
        for j in range(8):
            pt = ppw.tile([128, 128], BF, name="psw", tag="psw")
            nc.tensor.transpose(pt[0:128, 0:BPC],
                                hdn[:, j * 128:(j + 1) * 128],
                                idtb[0:BPC, 0:BPC])
            nc.vector.tensor_copy(out=hT[:, j * BPC:(j + 1) * BPC],
                                  in_=pt[:, 0:BPC])
        pso = ppw.tile([128, 512], DT, name="psw", tag="psw")
        for j in range(8):
            nc.tensor.matmul(pso[0:BPC, :], hT[:, j * BPC:(j + 1) * BPC],
                             wa2[j], start=(j == 0), stop=(j == 7))
        osb = pfin.tile([BPC, D], DT, name="osb", tag="osb")
        nc.vector.tensor_add(osb, pso[0:BPC, :], b2b)
        nc.sync.dma_start(out=out[:, :], in_=osb)

    nc.finalize()
    return nc


_NC = {}


def _get_module(rep: int = 1):
    if rep not in _NC:
        _NC[rep] = build_module(rep)
    return _NC[rep]


def make_in_maps(inputs):
    E4 = ml_dtypes.float8_e4m3
    BF16 = ml_dtypes.bfloat16
    f32 = lambda a: np.ascontiguousarray(np.asarray(a, dtype=np.float32))
    bf = lambda a: np.ascontiguousarray(np.asarray(a, dtype=np.float32)
                                        .astype(BF16))
    masks = build_masks().astype(E4)
    identb = np.eye(128, dtype=BF16)
    ident8 = np.eye(128, dtype=np.float32).astype(E4)
    idgs8 = (GS * np.eye(128, dtype=np.float32)).astype(E4)

    def pack_pairs(wmat, scale):
        """[D, F] f32 -> [2, 128, 2, F] fp8: d-chunk pairs interleaved."""
        w4 = (np.asarray(wmat, np.float32) * scale).reshape(4, 128, -1)
        return np.ascontiguousarray(
            np.stack([np.stack([w4[2 * p], w4[2 * p + 1]], axis=1)
                      for p in range(2)])).astype(E4)

    wv8 = pack_pairs(inputs["Wv"], WS)
    wh8 = np.stack([pack_pairs(inputs["W_hop"][h], WS) for h in range(H)])
    w0s = GS * (np.asarray(inputs["Wk"], np.float32)
                @ np.asarray(inputs["bq"], np.float32))

    shared = {
        "Wq": bf(inputs["Wq"]), "Wk": bf(inputs["Wk"]),
        "wv8": wv8, "wh8": wh8, "w0s": f32(w0s),
        # bv pre-scaled by AS: attended evac computes AS*psum + (AS*bv)
        "bv": f32(AS * np.asarray(inputs["bv"], np.float32)),
        "b_hop": f32(inputs["b_hop"]),
        "W_agg1": bf(inputs["W_agg1"]), "b_agg1": f32(inputs["b_agg1"]),
        "W_agg2": bf(inputs["W_agg2"]), "b_agg2": f32(inputs["b_agg2"]),
        "masks": masks, "identb": identb, "ident8": ident8, "idgs8": idgs8,
    }
    in_maps = []
    for c in range(N_CORES):
        sl = slice(c * BPC, (c + 1) * BPC)
        perm = lambda a: bf(np.asarray(a)[:, :, sl, :]
                            .transpose(2, 1, 0, 3).reshape(NT, D))
        in_maps.append({
            **shared,
            "what": perm(inputs["what"]),
            "action": perm(inputs["action"]),
            "result": perm(inputs["result"]),
        })
    return in_maps


def kernel(**inputs) -> np.ndarray:
    nc = _get_module()
    res = run_bass_kernel_spmd(nc, make_in_maps(inputs),
                               core_ids=list(range(N_CORES)))
    return np.concatenate([res.results[c]["out"] for c in range(N_CORES)],
                          axis=0)
